# revision 5
# baseline (speedup 1.0000x reference)
"""Trainium2 Bass kernel for the stacked-LSTM model (nn_Model2_16904991277618).

Model: LSTM-A(64->40, return_sequences) -> LSTM-B(40->40, last) over T=1024,
plus a small dense tail on `feat`, concat, 3 dense layers -> sigmoid [B,1].

Strategy (v3): data-parallel over batch (B=512 -> 64 rows/core),
feature-major layout. Cells A (step t) and B (step t-2) run as TWO
INDEPENDENT dependency chains per macro-step so their serial latencies hide
each other; only the cell-state Tanh is shared (single join point).

Per macro-step:
  PE : A = 2 matmuls (K=128 rhs packs h + bias-ones + x in one tile);
       B = 4 matmuls (hB+bias from a small ring, hA read in place from the
       staged x chunk where the A-chain wrote it -- zero copies).
  Act: sigma1A, sigma1B (all 4 gates per cell in one Sigmoid; the g-gate is
       pre-scaled by 2 so tanh(g) = 2*sig(2g)-1 comes for free), and one
       fused Tanh(scale=2) over both cells' half-cell-states.
  DVE: m2 = (sg-.5)*si (fused scalar_tensor_tensor), cn = m2 + p,
       h = so*tc -- per cell, all bf16 SBUF.
  Pool(gpsimd): p = sf * c_half per cell (offloaded to the idle engine).

Gate bands (SBUF-SBUF DVE operand pairs must share partition base; output
base may differ): f,o at rows 0:40; i,g at rows 64:104.
rhs rows: 0:40 = h (recurrent), 40 = ones (bias), 64:128 = x.
Half-cell-state trick: state C = c/2, so C' = sf*C + (sg2-0.5)*si exactly,
and tanh(c) = Tanh(C, scale=2).
"""

import functools
import os
import sys

import numpy as np

for _p in ("/opt/trn_rl_repo", "/root/.axon_site/_ro/trn_rl_repo"):
    if os.path.isdir(_p) and _p not in sys.path:
        sys.path.insert(0, _p)

import ml_dtypes  # noqa: E402

import concourse.bass as bass  # noqa: E402
import concourse.bacc as bacc  # noqa: E402
import concourse.mybir as mybir  # noqa: E402
import concourse.tile as tile  # noqa: E402
from concourse.bass_utils import run_bass_kernel_spmd  # noqa: E402

F32 = mybir.dt.float32
BF16 = mybir.dt.bfloat16
AF = mybir.ActivationFunctionType
OP = mybir.AluOpType

NCORES = 8
H = 40
D = 10
F = 64

# gate column ranges in the reference [*, 4H] weight matrices
_I, _Fg, _G, _O = slice(0, 40), slice(40, 80), slice(80, 120), slice(120, 160)


def _bf(x):
    return np.ascontiguousarray(x, dtype=ml_dtypes.bfloat16)


def _f32c(x):
    return np.ascontiguousarray(x, dtype=np.float32)


def _gpair(Wtop, Wbot, rows, scale_bot=1.0):
    """lhsT [rows, 128] with gate-weight block Wtop at cols 0:40 and
    Wbot*scale_bot at cols 64:104. W* are [k, 40] (k <= rows)."""
    out = np.zeros((rows, 128), np.float32)
    k = Wtop.shape[0]
    out[0:k, 0:40] = Wtop
    out[0:k, 64:104] = scale_bot * Wbot
    return out


def _build_program(T, BC):
    CHUNK_T = min(T, 128)
    n_chunks = T // CHUNK_T
    assert n_chunks * CHUNK_T == T
    BC2 = 2 * BC
    NRING = 4

    nc = bacc.Bacc("TRN2", debug=False, target_bir_lowering=False,
                   num_devices=NCORES)

    def din(name, shape, dt):
        return nc.dram_tensor(name, list(shape), dt, kind="ExternalInput").ap()

    xt = din("xt", (F, T * BC), BF16)
    featT = din("featT", (F, BC), BF16)
    d_in = {
        "wa_fi": din("wa_fi", (128, 128), BF16),    # rows: h|bias|0|x
        "wa_og": din("wa_og", (128, 128), BF16),
        "wbr_fi": din("wbr_fi", (41, 128), BF16),   # rows: hB|bias
        "wbr_og": din("wbr_og", (41, 128), BF16),
        "wbk_fi": din("wbk_fi", (40, 128), BF16),   # rows: hA
        "wbk_og": din("wbk_og", (40, 128), BF16),
        "wg": din("wg", (F, D), BF16),
        "wh": din("wh", (D, D), BF16),
        "wc": din("wc", (74, 2 * D), BF16),
        "wd": din("wd", (2 * D, D), BF16),
        "wo": din("wo", (D, 1), BF16),
        "bg": din("bg", (D, 1), F32),
        "bh": din("bh", (D, 1), F32),
        "bc2": din("bc2", (2 * D, 1), F32),
        "bd": din("bd", (D, 1), F32),
        "bo": din("bo", (1, 1), F32),
    }

    out_dram = nc.dram_tensor("out", [1, BC], F32, kind="ExternalOutput").ap()

    from contextlib import ExitStack

    with tile.TileContext(nc) as tc:
        with ExitStack() as ctx:
            wpool = ctx.enter_context(tc.tile_pool(name="w", bufs=1))
            xpool = ctx.enter_context(tc.tile_pool(name="x", bufs=1))
            gpool = ctx.enter_context(tc.tile_pool(name="g", bufs=3))
            cpool = ctx.enter_context(tc.tile_pool(name="c", bufs=3))
            tpool = ctx.enter_context(tc.tile_pool(name="t", bufs=3))
            spool = ctx.enter_context(tc.tile_pool(name="s", bufs=1))
            psum = ctx.enter_context(tc.tile_pool(name="ps", bufs=4,
                                                  space="PSUM"))

            W = {}
            for nm, src in d_in.items():
                t_ = wpool.tile(list(src.shape), src.dtype, name=f"w_{nm}")
                nc.sync.dma_start(t_[:], src[:])
                W[nm] = t_
            ftile = wpool.tile([F, BC], BF16, name="w_featT")
            nc.sync.dma_start(ftile[:], featT[:])

            # x chunks: rows 64:128 = x features (DMA), row 40 = ones,
            # rows 0:40 = hA(t-1), written on-chip by the A-chain.
            xch = []
            for ci in range(n_chunks):
                xc = xpool.tile([128, CHUNK_T * BC], BF16, name=f"xc{ci}",
                                tag=f"xc{ci}")
                nc.sync.dma_start(
                    xc[64:128, :],
                    xt[:, ci * CHUNK_T * BC:(ci + 1) * CHUNK_T * BC])
                # ones row is partition 40; engine APs need 32-aligned bases,
                # so set rows 32:64 (41:63 unused, 32:40 rewritten by hA
                # before any matmul reads them)
                nc.gpsimd.memset(xc[32:64, :], 1.0)
                xch.append(xc)
            # t=0 block: hA(-1) = 0, keep the ones row
            nc.gpsimd.memset(xch[0][0:32, 0:BC], 0.0)
            nc.gpsimd.memset(xch[0][32:40, 0:BC], 0.0)

            # hB ring: rows 0:40 = hB, row 40 = ones
            ring = []
            for k in range(NRING):
                rb = xpool.tile([64, BC], BF16, name=f"rb{k}", tag=f"rb{k}")
                nc.gpsimd.memset(rb[0:32, :], 0.0)
                nc.gpsimd.memset(rb[32:64, :], 1.0)
                nc.gpsimd.memset(rb[32:40, :], 0.0)
                ring.append(rb)

            hsc = wpool.tile([64, BC], BF16, name="hsc")  # hA(T-1) scratch
            zcat = spool.tile([74, BC], BF16, name="zcat")
            nc.gpsimd.memset(zcat[:], 0.0)

            cA_prev = cpool.tile([40, BC], BF16, name="cA_init", tag="cA")
            nc.gpsimd.memset(cA_prev[:], 0.0)
            cB_prev = cpool.tile([40, BC], BF16, name="cB_init", tag="cB")
            nc.gpsimd.memset(cB_prev[:], 0.0)

            for s in range(T + 2):
                t, tau = s, s - 2
                A = t < T
                Bact = tau >= 0

                gpA = gpB = None
                if A:
                    ci, tl = divmod(t, CHUNK_T)
                    rhsA = xch[ci][0:128, tl * BC:(tl + 1) * BC]
                    zzA = psum.tile([128, BC2], F32, name=f"zzA{s}",
                                    tag="zzA")
                    nc.tensor.matmul(zzA[:, 0:BC], W["wa_fi"][:], rhsA,
                                     start=True, stop=True)
                    nc.tensor.matmul(zzA[:, BC:BC2], W["wa_og"][:], rhsA,
                                     start=True, stop=True)
                    gpA = gpool.tile([128, BC2], BF16, name=f"gpA{s}",
                                     tag="gpA")
                    nc.scalar.activation(gpA[:], zzA[:], AF.Sigmoid)
                if Bact:
                    # hA(tau) sits in the staged x chunk at block tau+1
                    # (written there by the A-chain), hB(tau-1) in the ring
                    if tau + 1 < T:
                        ci2, tl2 = divmod(tau + 1, CHUNK_T)
                        rhsBk = xch[ci2][0:40, tl2 * BC:(tl2 + 1) * BC]
                    else:
                        rhsBk = hsc[0:40, :]
                    rhsBr = ring[tau % NRING][0:41, :]
                    zzB = psum.tile([128, BC2], F32, name=f"zzB{s}",
                                    tag="zzB")
                    nc.tensor.matmul(zzB[:, 0:BC], W["wbr_fi"][:], rhsBr,
                                     start=True, stop=False)
                    nc.tensor.matmul(zzB[:, 0:BC], W["wbk_fi"][:], rhsBk,
                                     start=False, stop=True)
                    nc.tensor.matmul(zzB[:, BC:BC2], W["wbr_og"][:], rhsBr,
                                     start=True, stop=False)
                    nc.tensor.matmul(zzB[:, BC:BC2], W["wbk_og"][:], rhsBk,
                                     start=False, stop=True)
                    gpB = gpool.tile([128, BC2], BF16, name=f"gpB{s}",
                                     tag="gpB")
                    nc.scalar.activation(gpB[:], zzB[:], AF.Sigmoid)

                # c-updates: m2 = (sg2-0.5)*si on DVE, p = sf*C on gpsimd,
                # C' = m2 + p on DVE.  cn holds [A | B] halves for the
                # shared Tanh.
                cn = cpool.tile([40, BC2], BF16, name=f"cn{s}", tag="cn")
                tcf = tpool.tile([40, BC2], BF16, name=f"tc{s}", tag="tc")
                if A:
                    m2A = tpool.tile([40, BC], BF16, name=f"m2A{s}",
                                     tag="m2A")
                    pA = tpool.tile([40, BC], BF16, name=f"pA{s}", tag="pA")
                    nc.vector.scalar_tensor_tensor(
                        m2A[:], gpA[64:104, BC:BC2], -0.5,
                        gpA[64:104, 0:BC], OP.add, OP.mult)
                    nc.gpsimd.tensor_tensor(pA[:], gpA[0:40, 0:BC],
                                            cA_prev[:], OP.mult)
                    nc.vector.tensor_tensor(cn[:, 0:BC], m2A[:], pA[:],
                                            OP.add)
                    cA_prev = cn[:, 0:BC]
                if Bact:
                    m2B = tpool.tile([40, BC], BF16, name=f"m2B{s}",
                                     tag="m2B")
                    pB = tpool.tile([40, BC], BF16, name=f"pB{s}", tag="pB")
                    nc.vector.scalar_tensor_tensor(
                        m2B[:], gpB[64:104, BC:BC2], -0.5,
                        gpB[64:104, 0:BC], OP.add, OP.mult)
                    nc.gpsimd.tensor_tensor(pB[:], gpB[0:40, 0:BC],
                                            cB_prev[:], OP.mult)
                    nc.vector.tensor_tensor(cn[:, BC:BC2], m2B[:], pB[:],
                                            OP.add)
                    cB_prev = cn[:, BC:BC2]

                nc.scalar.activation(tcf[:], cn[:], AF.Tanh, scale=2.0)

                if A:
                    if t + 1 < T:
                        ci3, tl3 = divmod(t + 1, CHUNK_T)
                        hA_dst = xch[ci3][0:40, tl3 * BC:(tl3 + 1) * BC]
                    else:
                        hA_dst = hsc[0:40, :]
                    nc.vector.tensor_tensor(hA_dst, gpA[0:40, BC:BC2],
                                            tcf[:, 0:BC], OP.mult)
                if Bact:
                    hB_dst = (zcat[0:40, :] if s == T + 1
                              else ring[(tau + 1) % NRING][0:40, :])
                    nc.vector.tensor_tensor(hB_dst, gpB[0:40, BC:BC2],
                                            tcf[:, BC:BC2], OP.mult)

            # ---- dense tail ----
            ps1 = psum.tile([D, BC], F32, name="ps1", tag="zzA")
            nc.tensor.matmul(ps1[:], W["wg"][:], ftile[:],
                             start=True, stop=True)
            y1 = spool.tile([D, BC], BF16, name="y1")
            nc.scalar.activation(y1[:], ps1[:], AF.Tanh, bias=W["bg"][:])

            ps2 = psum.tile([D, BC], F32, name="ps2", tag="zzB")
            nc.tensor.matmul(ps2[:], W["wh"][:], y1[:], start=True, stop=True)
            nc.scalar.activation(zcat[64:74, :], ps2[:], AF.Tanh,
                                 bias=W["bh"][:])

            ps3 = psum.tile([2 * D, BC], F32, name="ps3", tag="zzA")
            nc.tensor.matmul(ps3[:], W["wc"][:], zcat[:], start=True,
                             stop=True)
            c1 = spool.tile([2 * D, BC], BF16, name="c1")
            nc.scalar.activation(c1[:], ps3[:], AF.Relu, bias=W["bc2"][:])

            ps4 = psum.tile([D, BC], F32, name="ps4", tag="zzB")
            nc.tensor.matmul(ps4[:], W["wd"][:], c1[:], start=True, stop=True)
            d1 = spool.tile([D, BC], BF16, name="d1")
            nc.scalar.activation(d1[:], ps4[:], AF.Relu, bias=W["bd"][:])

            ps5 = psum.tile([1, BC], F32, name="ps5", tag="zzA")
            nc.tensor.matmul(ps5[:], W["wo"][:], d1[:], start=True, stop=True)
            osb = spool.tile([1, BC], F32, name="osb")
            nc.scalar.activation(osb[:], ps5[:], AF.Sigmoid, bias=W["bo"][:])

            nc.sync.dma_start(out_dram[:], osb[:])

    nc.compile()
    return nc


@functools.lru_cache(maxsize=2)
def _program(T, BC):
    return _build_program(T, BC)


def _prep_shared(Wa_k, Wa_r, ba, Wb_k, Wb_r, bb, Wg, bg, Wh, bh, Wc, bc, Wd,
                 bd, Wo, bo):
    Wa_k, Wa_r, ba = (np.asarray(a, np.float32) for a in (Wa_k, Wa_r, ba))
    Wb_k, Wb_r, bb = (np.asarray(a, np.float32) for a in (Wb_k, Wb_r, bb))

    def a_pack(s_top, s_bot, sc):
        out = np.zeros((128, 128), np.float32)
        out[0:40] = _gpair(Wa_r[:, s_top], Wa_r[:, s_bot], 40, sc)
        out[40, 0:40] = ba[s_top]
        out[40, 64:104] = sc * ba[s_bot]
        out[64:128] = _gpair(Wa_k[:, s_top], Wa_k[:, s_bot], 64, sc)
        return _bf(out)

    def br_pack(s_top, s_bot, sc):
        out = np.zeros((41, 128), np.float32)
        out[0:40] = _gpair(Wb_r[:, s_top], Wb_r[:, s_bot], 40, sc)
        out[40, 0:40] = bb[s_top]
        out[40, 64:104] = sc * bb[s_bot]
        return _bf(out)

    def bk_pack(s_top, s_bot, sc):
        return _bf(_gpair(Wb_k[:, s_top], Wb_k[:, s_bot], 40, sc))

    wc_re = np.zeros((74, 2 * D), np.float32)
    wc_re[0:40] = np.asarray(Wc, np.float32)[0:40]
    wc_re[64:74] = np.asarray(Wc, np.float32)[40:50]
    return {
        "wa_fi": a_pack(_Fg, _I, 1.0),
        "wa_og": a_pack(_O, _G, 2.0),
        "wbr_fi": br_pack(_Fg, _I, 1.0),
        "wbr_og": br_pack(_O, _G, 2.0),
        "wbk_fi": bk_pack(_Fg, _I, 1.0),
        "wbk_og": bk_pack(_O, _G, 2.0),
        "wg": _bf(Wg), "wh": _bf(Wh), "wc": _bf(wc_re), "wd": _bf(Wd),
        "wo": _bf(Wo),
        "bg": _f32c(np.asarray(bg)[:, None]),
        "bh": _f32c(np.asarray(bh)[:, None]),
        "bc2": _f32c(np.asarray(bc)[:, None]),
        "bd": _f32c(np.asarray(bd)[:, None]),
        "bo": _f32c(np.asarray(bo)[:, None]),
    }


def _prep_seq(seq, T, BC):
    # [core, F, T*BC]: row f, col t*BC + b
    arr = np.asarray(seq, np.float32).reshape(NCORES, BC, T, F)
    arr = arr.transpose(0, 3, 2, 1).reshape(NCORES, F, T * BC)
    return _bf(arr)


def kernel(seq, feat, Wa_k, Wa_r, ba, Wb_k, Wb_r, bb, Wg, bg, Wh, bh, Wc, bc,
           Wd, bd, Wo, bo, _trace=False):
    seq = np.asarray(seq)
    feat = np.asarray(feat)
    B, T, _ = seq.shape
    assert B % NCORES == 0
    BC = B // NCORES

    nc = _program(T, BC)

    shared = _prep_shared(Wa_k, Wa_r, ba, Wb_k, Wb_r, bb, Wg, bg, Wh, bh, Wc,
                          bc, Wd, bd, Wo, bo)
    xt = _prep_seq(seq, T, BC)
    featc = np.asarray(feat, np.float32).reshape(NCORES, BC, F)

    in_maps = []
    for c in range(NCORES):
        m = dict(shared)
        m["xt"] = xt[c]
        m["featT"] = _bf(featc[c].T)
        in_maps.append(m)

    res = run_bass_kernel_spmd(nc, in_maps, core_ids=list(range(NCORES)),
                               trace=_trace)
    out = np.concatenate([res.results[c]["out"][0] for c in range(NCORES)])
    out = out.astype(np.float32).reshape(B, 1)
    if _trace:
        kernel.last_results = res
    return out


# revision 10
# speedup vs baseline: 1.2593x; 1.2593x over previous
"""Trainium2 Bass kernel for the stacked-LSTM model (nn_Model2_16904991277618).

Model: LSTM-A(64->40, return_sequences) -> LSTM-B(40->40, last) over T=1024,
plus a small dense tail on `feat`, concat, 3 dense layers -> sigmoid [B,1].

Strategy (v4): data-parallel over batch (B=512 -> 64 rows/core),
feature-major layout. Wall-clock is T x (per-step serial chain latency of
cell A), so everything is organized to minimize that chain:

  A-chain per step: h-matmul (x-part prefetched a step early into the same
  PSUM accumulation group) -> one Sigmoid over all 4 gates (g pre-scaled by
  2 so tanh(g)=2*sig(2g)-1 is free) -> 3 back-to-back DVE ops
  (m2=(sg-.5)*si via fused scalar_tensor_tensor, p=sf*C, C'=m2+p)
  -> Tanh(scale=2) -> h=so*tc -> next h-matmul.

Cell B (step t-2) runs the same pipeline as an independent chain with two
macro-steps of slack; its ops are queued behind the A-chain ops on each
engine so they fill idle windows without delaying A. B reads hA in place
from the staged x chunk (where the A-chain wrote it) -- zero copies.

Gate bands (SBUF-SBUF DVE operand pairs must share partition base; output
base may differ): f,o at rows 0:40; i,g at rows 64:104.
rhs rows: 0:40 = h (recurrent), 32 = ones (bias), 64:128 = x.
Half-cell-state trick: state C = c/2, so C' = sf*C + (sg2-0.5)*si exactly,
and tanh(c) = Tanh(C, scale=2).
"""

import functools
import os
import sys

import numpy as np

for _p in ("/opt/trn_rl_repo", "/root/.axon_site/_ro/trn_rl_repo"):
    if os.path.isdir(_p) and _p not in sys.path:
        sys.path.insert(0, _p)

import ml_dtypes  # noqa: E402

import concourse.bass as bass  # noqa: E402
import concourse.bacc as bacc  # noqa: E402
import concourse.mybir as mybir  # noqa: E402
import concourse.tile as tile  # noqa: E402
from concourse.bass_utils import run_bass_kernel_spmd  # noqa: E402

F32 = mybir.dt.float32
BF16 = mybir.dt.bfloat16
AF = mybir.ActivationFunctionType
OP = mybir.AluOpType

NCORES = 8
H = 40
D = 10
F = 64

# gate column ranges in the reference [*, 4H] weight matrices
_I, _Fg, _G, _O = slice(0, 40), slice(40, 80), slice(80, 120), slice(120, 160)


def _bf(x):
    return np.ascontiguousarray(x, dtype=ml_dtypes.bfloat16)


def _f32c(x):
    return np.ascontiguousarray(x, dtype=np.float32)


def _gpair(Wtop, Wbot, rows, scale_bot=1.0):
    """lhsT [rows, 128]: gate block Wtop at cols 0:40, Wbot*scale_bot at
    cols 64:104. W* are [k, 40] with k <= rows."""
    out = np.zeros((rows, 128), np.float32)
    k = Wtop.shape[0]
    out[0:k, 0:40] = Wtop
    out[0:k, 64:104] = scale_bot * Wbot
    return out


def _build_program(T, BC):
    CHUNK_T = min(T, 128)
    n_chunks = T // CHUNK_T
    assert n_chunks * CHUNK_T == T
    BC2 = 2 * BC
    NRING = 4

    nc = bacc.Bacc("TRN2", debug=False, target_bir_lowering=False,
                   num_devices=NCORES)

    def din(name, shape, dt):
        return nc.dram_tensor(name, list(shape), dt, kind="ExternalInput").ap()

    xt = din("xt", (F, T * BC), BF16)
    featT = din("featT", (F, BC), BF16)
    d_in = {
        # A-cell: one lhsT per gate-pair, rows = xc rows 0:128
        # (hA | ones | zeros | x)
        "wa_fi": din("wa_fi", (128, 128), BF16),
        "wa_og": din("wa_og", (128, 128), BF16),
        # B-cell: hB + bias from ring, hA from the staged x chunk
        "wbr_fi": din("wbr_fi", (41, 128), BF16),
        "wbr_og": din("wbr_og", (41, 128), BF16),
        "wbk_fi": din("wbk_fi", (40, 128), BF16),
        "wbk_og": din("wbk_og", (40, 128), BF16),
        "wg": din("wg", (F, D), BF16),
        "wh": din("wh", (D, D), BF16),
        "wc": din("wc", (74, 2 * D), BF16),
        "wd": din("wd", (2 * D, D), BF16),
        "wo": din("wo", (D, 1), BF16),
        "bg": din("bg", (D, 1), F32),
        "bh": din("bh", (D, 1), F32),
        "bc2": din("bc2", (2 * D, 1), F32),
        "bd": din("bd", (D, 1), F32),
        "bo": din("bo", (1, 1), F32),
    }

    out_dram = nc.dram_tensor("out", [1, BC], F32, kind="ExternalOutput").ap()

    from contextlib import ExitStack

    with tile.TileContext(nc) as tc:
        with ExitStack() as ctx:
            wpool = ctx.enter_context(tc.tile_pool(name="w", bufs=1))
            xpool = ctx.enter_context(tc.tile_pool(name="x", bufs=1))
            gpool = ctx.enter_context(tc.tile_pool(name="g", bufs=3))
            cpool = ctx.enter_context(tc.tile_pool(name="c", bufs=3))
            tpool = ctx.enter_context(tc.tile_pool(name="t", bufs=3))
            spool = ctx.enter_context(tc.tile_pool(name="s", bufs=1))
            psum = ctx.enter_context(tc.tile_pool(name="ps", bufs=3,
                                                  space="PSUM"))

            W = {}
            for nm, src in d_in.items():
                t_ = wpool.tile(list(src.shape), src.dtype, name=f"w_{nm}")
                nc.sync.dma_start(t_[:], src[:])
                W[nm] = t_
            ftile = wpool.tile([F, BC], BF16, name="w_featT")
            nc.sync.dma_start(ftile[:], featT[:])

            # x chunks: rows 64:128 = x features (DMA), row 40 = ones,
            # rows 0:40 = hA(t-1) written per step by the A-chain (so each
            # gate-pair needs just ONE matmul -- no PSUM accumulation
            # groups, which must not interleave).
            xch = []
            for ci in range(n_chunks):
                xc = xpool.tile([128, CHUNK_T * BC], BF16, name=f"xc{ci}",
                                tag=f"xc{ci}")
                nc.sync.dma_start(
                    xc[64:128, :],
                    xt[:, ci * CHUNK_T * BC:(ci + 1) * CHUNK_T * BC])
                nc.gpsimd.memset(xc[32:64, :], 1.0)
                xch.append(xc)
            nc.gpsimd.memset(xch[0][0:32, 0:BC], 0.0)
            nc.gpsimd.memset(xch[0][32:40, 0:BC], 0.0)
            hsc = xpool.tile([40, BC], BF16, name="hsc")  # hA(T-1)

            # hB ring: rows 0:40 = hB, row 32... ones row shares the hA
            # convention: row 32 = ones after hB rows are rewritten
            ring = []
            for k in range(NRING):
                rb = xpool.tile([64, BC], BF16, name=f"rb{k}", tag=f"rb{k}")
                nc.gpsimd.memset(rb[0:32, :], 0.0)
                nc.gpsimd.memset(rb[32:64, :], 1.0)
                nc.gpsimd.memset(rb[32:40, :], 0.0)
                ring.append(rb)

            zcat = spool.tile([74, BC], BF16, name="zcat")
            nc.gpsimd.memset(zcat[:], 0.0)

            cA_prev = cpool.tile([40, BC], BF16, name="cA_init", tag="cA")
            nc.gpsimd.memset(cA_prev[:], 0.0)
            cB_prev = cpool.tile([40, BC], BF16, name="cB_init", tag="cB")
            nc.gpsimd.memset(cB_prev[:], 0.0)

            def a_rhs(step):
                ci, tl = divmod(step, CHUNK_T)
                return xch[ci], tl * BC, (tl + 1) * BC

            for s in range(T + 2):
                t, tau = s, s - 2
                A = t < T
                Bact = tau >= 0

                # ---- A-chain critical ops first ----
                gpA = gpB = None
                if A:
                    zzA = psum.tile([128, BC2], F32, name=f"zzA{s}",
                                    tag="zzA")
                    xci, cia, cib = a_rhs(t)
                    rhsA = xci[0:128, cia:cib]
                    nc.tensor.matmul(zzA[:, 0:BC], W["wa_fi"][:], rhsA,
                                     start=True, stop=True)
                    nc.tensor.matmul(zzA[:, BC:BC2], W["wa_og"][:], rhsA,
                                     start=True, stop=True)
                    gpA = gpool.tile([128, BC2], BF16, name=f"gpA{s}",
                                     tag="gpA")
                    nc.scalar.activation(gpA[:], zzA[:], AF.Sigmoid)

                    m2A = tpool.tile([40, BC], BF16, name=f"m2A{s}",
                                     tag="m2A")
                    pA = tpool.tile([40, BC], BF16, name=f"pA{s}", tag="pA")
                    cnA = cpool.tile([40, BC], BF16, name=f"cnA{s}",
                                     tag="cA")
                    tcA = tpool.tile([40, BC], BF16, name=f"tcA{s}",
                                     tag="tcA")
                    nc.vector.scalar_tensor_tensor(
                        m2A[:], gpA[64:104, BC:BC2], -0.5,
                        gpA[64:104, 0:BC], OP.add, OP.mult)
                    nc.vector.tensor_tensor(pA[:], gpA[0:40, 0:BC],
                                            cA_prev[:], OP.mult)
                    nc.vector.tensor_tensor(cnA[:], m2A[:], pA[:], OP.add)
                    cA_prev = cnA
                    nc.scalar.activation(tcA[:], cnA[:], AF.Tanh, scale=2.0)
                    if t + 1 < T:
                        xcn, cna, cnb = a_rhs(t + 1)
                        hA_dst = xcn[0:40, cna:cnb]
                    else:
                        hA_dst = hsc[:]
                    nc.vector.tensor_tensor(hA_dst, gpA[0:40, BC:BC2],
                                            tcA[:, :], OP.mult)

                # ---- B-chain (two steps of slack) ----
                if Bact:
                    if tau + 1 < T:
                        xcb, bka, bkb = a_rhs(tau + 1)
                        rhsBk = xcb[0:40, bka:bkb]
                    else:
                        rhsBk = hsc[:]
                    rhsBr = ring[tau % NRING][0:41, :]
                    zzB = psum.tile([128, BC2], F32, name=f"zzB{s}",
                                    tag="zzB")
                    nc.tensor.matmul(zzB[:, 0:BC], W["wbr_fi"][:], rhsBr,
                                     start=True, stop=False)
                    nc.tensor.matmul(zzB[:, 0:BC], W["wbk_fi"][:], rhsBk,
                                     start=False, stop=True)
                    nc.tensor.matmul(zzB[:, BC:BC2], W["wbr_og"][:], rhsBr,
                                     start=True, stop=False)
                    nc.tensor.matmul(zzB[:, BC:BC2], W["wbk_og"][:], rhsBk,
                                     start=False, stop=True)
                    gpB = gpool.tile([128, BC2], BF16, name=f"gpB{s}",
                                     tag="gpB")
                    nc.scalar.activation(gpB[:], zzB[:], AF.Sigmoid)

                    m2B = tpool.tile([40, BC], BF16, name=f"m2B{s}",
                                     tag="m2B")
                    pB = tpool.tile([40, BC], BF16, name=f"pB{s}", tag="pB")
                    cnB = cpool.tile([40, BC], BF16, name=f"cnB{s}",
                                     tag="cB")
                    tcB = tpool.tile([40, BC], BF16, name=f"tcB{s}",
                                     tag="tcB")
                    nc.vector.scalar_tensor_tensor(
                        m2B[:], gpB[64:104, BC:BC2], -0.5,
                        gpB[64:104, 0:BC], OP.add, OP.mult)
                    nc.vector.tensor_tensor(pB[:], gpB[0:40, 0:BC],
                                            cB_prev[:], OP.mult)
                    nc.vector.tensor_tensor(cnB[:], m2B[:], pB[:], OP.add)
                    cB_prev = cnB
                    nc.scalar.activation(tcB[:], cnB[:], AF.Tanh, scale=2.0)
                    hB_dst = (zcat[0:40, :] if s == T + 1
                              else ring[(tau + 1) % NRING][0:40, :])
                    nc.vector.tensor_tensor(hB_dst, gpB[0:40, BC:BC2],
                                            tcB[:, :], OP.mult)

            # ---- dense tail ----
            ps1 = psum.tile([D, BC], F32, name="ps1", tag="zzA")
            nc.tensor.matmul(ps1[:], W["wg"][:], ftile[:],
                             start=True, stop=True)
            y1 = spool.tile([D, BC], BF16, name="y1")
            nc.scalar.activation(y1[:], ps1[:], AF.Tanh, bias=W["bg"][:])

            ps2 = psum.tile([D, BC], F32, name="ps2", tag="zzB")
            nc.tensor.matmul(ps2[:], W["wh"][:], y1[:], start=True, stop=True)
            nc.scalar.activation(zcat[64:74, :], ps2[:], AF.Tanh,
                                 bias=W["bh"][:])

            ps3 = psum.tile([2 * D, BC], F32, name="ps3", tag="zzA")
            nc.tensor.matmul(ps3[:], W["wc"][:], zcat[:], start=True,
                             stop=True)
            c1 = spool.tile([2 * D, BC], BF16, name="c1")
            nc.scalar.activation(c1[:], ps3[:], AF.Relu, bias=W["bc2"][:])

            ps4 = psum.tile([D, BC], F32, name="ps4", tag="zzB")
            nc.tensor.matmul(ps4[:], W["wd"][:], c1[:], start=True, stop=True)
            d1 = spool.tile([D, BC], BF16, name="d1")
            nc.scalar.activation(d1[:], ps4[:], AF.Relu, bias=W["bd"][:])

            ps5 = psum.tile([1, BC], F32, name="ps5", tag="zzA")
            nc.tensor.matmul(ps5[:], W["wo"][:], d1[:], start=True, stop=True)
            osb = spool.tile([1, BC], F32, name="osb")
            nc.scalar.activation(osb[:], ps5[:], AF.Sigmoid, bias=W["bo"][:])

            nc.sync.dma_start(out_dram[:], osb[:])

    nc.compile()
    return nc


@functools.lru_cache(maxsize=2)
def _program(T, BC):
    return _build_program(T, BC)


def _prep_shared(Wa_k, Wa_r, ba, Wb_k, Wb_r, bb, Wg, bg, Wh, bh, Wc, bc, Wd,
                 bd, Wo, bo):
    Wa_k, Wa_r, ba = (np.asarray(a, np.float32) for a in (Wa_k, Wa_r, ba))
    Wb_k, Wb_r, bb = (np.asarray(a, np.float32) for a in (Wb_k, Wb_r, bb))

    def a_pack(s_top, s_bot, sc):
        # lhsT rows map to xc rows 0:128: 0:40 = hA, 40 = ones, 64:128 = x
        out = np.zeros((128, 128), np.float32)
        out[0:40] = _gpair(Wa_r[:, s_top], Wa_r[:, s_bot], 40, sc)
        out[40, 0:40] = ba[s_top]
        out[40, 64:104] = sc * ba[s_bot]
        out[64:128] = _gpair(Wa_k[:, s_top], Wa_k[:, s_bot], 64, sc)
        return _bf(out)

    def br_pack(s_top, s_bot, sc):
        out = np.zeros((41, 128), np.float32)
        out[0:40] = _gpair(Wb_r[:, s_top], Wb_r[:, s_bot], 40, sc)
        out[40, 0:40] = bb[s_top]
        out[40, 64:104] = sc * bb[s_bot]
        return _bf(out)

    def bk_pack(s_top, s_bot, sc):
        return _bf(_gpair(Wb_k[:, s_top], Wb_k[:, s_bot], 40, sc))

    wc_re = np.zeros((74, 2 * D), np.float32)
    wc_re[0:40] = np.asarray(Wc, np.float32)[0:40]
    wc_re[64:74] = np.asarray(Wc, np.float32)[40:50]
    return {
        "wa_fi": a_pack(_Fg, _I, 1.0),
        "wa_og": a_pack(_O, _G, 2.0),
        "wbr_fi": br_pack(_Fg, _I, 1.0),
        "wbr_og": br_pack(_O, _G, 2.0),
        "wbk_fi": bk_pack(_Fg, _I, 1.0),
        "wbk_og": bk_pack(_O, _G, 2.0),
        "wg": _bf(Wg), "wh": _bf(Wh), "wc": _bf(wc_re), "wd": _bf(Wd),
        "wo": _bf(Wo),
        "bg": _f32c(np.asarray(bg)[:, None]),
        "bh": _f32c(np.asarray(bh)[:, None]),
        "bc2": _f32c(np.asarray(bc)[:, None]),
        "bd": _f32c(np.asarray(bd)[:, None]),
        "bo": _f32c(np.asarray(bo)[:, None]),
    }


def _prep_seq(seq, T, BC):
    # [core, F, T*BC]: row f, col t*BC + b
    arr = np.asarray(seq, np.float32).reshape(NCORES, BC, T, F)
    arr = arr.transpose(0, 3, 2, 1).reshape(NCORES, F, T * BC)
    return _bf(arr)


def kernel(seq, feat, Wa_k, Wa_r, ba, Wb_k, Wb_r, bb, Wg, bg, Wh, bh, Wc, bc,
           Wd, bd, Wo, bo, _trace=False):
    seq = np.asarray(seq)
    feat = np.asarray(feat)
    B, T, _ = seq.shape
    assert B % NCORES == 0
    BC = B // NCORES

    nc = _program(T, BC)

    shared = _prep_shared(Wa_k, Wa_r, ba, Wb_k, Wb_r, bb, Wg, bg, Wh, bh, Wc,
                          bc, Wd, bd, Wo, bo)
    xt = _prep_seq(seq, T, BC)
    featc = np.asarray(feat, np.float32).reshape(NCORES, BC, F)

    in_maps = []
    for c in range(NCORES):
        m = dict(shared)
        m["xt"] = xt[c]
        m["featT"] = _bf(featc[c].T)
        in_maps.append(m)

    res = run_bass_kernel_spmd(nc, in_maps, core_ids=list(range(NCORES)),
                               trace=_trace)
    out = np.concatenate([res.results[c]["out"][0] for c in range(NCORES)])
    out = out.astype(np.float32).reshape(B, 1)
    if _trace:
        kernel.last_results = res
    return out
